# revision 4
# baseline (speedup 1.0000x reference)
"""Trainium2 Bass kernel for nn_CorrectorV2 (B=32768, H=256, 8 cores, data parallel).

Pipeline per core (Bc=4096 elements, batch-major <-> feature-major hybrid):
  1. preprocess (batch-major, DVE): data[b,63] = [gR, acc, ori] from aM/RMB
  2. PE-transpose data -> feature-major [63, n]
  3. x0 = relu(W1 @ data + b1)            (fp32 matmul, ACT relu+bias)
  4. 2x LSTM cell with zero init state -> only i,g,o gates needed (f is dead):
       h = sigmoid(o) * tanh(sigmoid(i) * tanh(g)),  g_pre = Wih x + bih + bhh
  5. out[b,10] = h2 @ W2.T + b2 (flipped matmul -> batch-major output)
  6. angle = atan2(sin_r, cos_r) via octant-reduced arctan; phi = angle*clip(weight+delta)
  7. rotate aM/wM/RMB about y by -phi (batch-major, DVE)

All matmuls are true fp32 (atan2 amplifies error by 1/|out| with |out|_min ~3e-4,
so reduced-precision matmuls are catastrophically wrong).
"""

import sys

if '/opt/trn_rl_repo' not in sys.path:
    sys.path.insert(0, '/opt/trn_rl_repo')

import numpy as np

import concourse.bass as bass
import concourse.bacc as bacc
import concourse.tile as tile
from concourse import mybir
from concourse.bass_utils import run_bass_kernel_spmd
from concourse.masks import make_identity
from concourse._compat import with_exitstack
from contextlib import ExitStack

F = mybir.ActivationFunctionType
A = mybir.AluOpType
FP32 = mybir.dt.float32

B = 32768
H = 256
N_CORES = 8
BC = B // N_CORES          # 4096 elements per core
T_ALL = BC // 128          # 32 chunks of 128
ST = 2                     # supertiles per core
TS = T_ALL // ST           # 16 chunks per supertile
NB = 512                   # LSTM batch tile
NBT = BC // NB             # 8 batch tiles
CPB = NB // 128            # 4 chunks per batch tile

_CACHE = {}


def _build():
    nc = bacc.Bacc("TRN2", target_bir_lowering=False, debug=False)

    # ---------------- DRAM tensors ----------------
    i_aM = nc.dram_tensor("aM", [BC, 6, 3], FP32, kind="ExternalInput").ap()
    i_wM = nc.dram_tensor("wM", [BC, 6, 3], FP32, kind="ExternalInput").ap()
    i_RMB = nc.dram_tensor("RMB", [BC, 6, 3, 3], FP32, kind="ExternalInput").ap()
    i_w = nc.dram_tensor("weight", [BC], FP32, kind="ExternalInput").ap()
    i_delta = nc.dram_tensor("delta", [1], FP32, kind="ExternalInput").ap()
    i_W1 = nc.dram_tensor("W1", [H, 63], FP32, kind="ExternalInput").ap()
    i_b1 = nc.dram_tensor("b1", [H], FP32, kind="ExternalInput").ap()
    i_Wih0 = nc.dram_tensor("Wih0", [4 * H, H], FP32, kind="ExternalInput").ap()
    i_bih0 = nc.dram_tensor("bih0", [4 * H], FP32, kind="ExternalInput").ap()
    i_bhh0 = nc.dram_tensor("bhh0", [4 * H], FP32, kind="ExternalInput").ap()
    i_Wih1 = nc.dram_tensor("Wih1", [4 * H, H], FP32, kind="ExternalInput").ap()
    i_bih1 = nc.dram_tensor("bih1", [4 * H], FP32, kind="ExternalInput").ap()
    i_bhh1 = nc.dram_tensor("bhh1", [4 * H], FP32, kind="ExternalInput").ap()
    i_W2 = nc.dram_tensor("W2", [10, H], FP32, kind="ExternalInput").ap()
    i_b2 = nc.dram_tensor("b2", [10], FP32, kind="ExternalInput").ap()
    o_aMc = nc.dram_tensor("aMc", [BC, 6, 3], FP32, kind="ExternalOutput").ap()
    o_wMc = nc.dram_tensor("wMc", [BC, 6, 3], FP32, kind="ExternalOutput").ap()
    o_RMBc = nc.dram_tensor("RMBc", [BC, 6, 3, 3], FP32, kind="ExternalOutput").ap()

    with tile.TileContext(nc) as tc:
        _emit(tc, i_aM, i_wM, i_RMB, i_w, i_delta, i_W1, i_b1,
              i_Wih0, i_bih0, i_bhh0, i_Wih1, i_bih1, i_bhh1, i_W2, i_b2,
              o_aMc, o_wMc, o_RMBc)
    nc.compile()
    return nc


@with_exitstack
def _emit(ctx: ExitStack, tc, i_aM, i_wM, i_RMB, i_w, i_delta, i_W1, i_b1,
          i_Wih0, i_bih0, i_bhh0, i_Wih1, i_bih1, i_bhh1, i_W2, i_b2,
          o_aMc, o_wMc, o_RMBc):
    nc = tc.nc
    consts = ctx.enter_context(tc.tile_pool(name="consts", bufs=1))
    stp = ctx.enter_context(tc.tile_pool(name="stp", bufs=2))       # supertile tiles
    btp = ctx.enter_context(tc.tile_pool(name="btp", bufs=2))       # batch-tile tiles
    angp = ctx.enter_context(tc.tile_pool(name="angp", bufs=2))     # angle/rot temps
    ps_tp = ctx.enter_context(tc.tile_pool(name="ps_tp", bufs=2, space="PSUM"))
    ps_x0 = ctx.enter_context(tc.tile_pool(name="ps_x0", bufs=2, space="PSUM"))
    ps_g = ctx.enter_context(tc.tile_pool(name="ps_g", bufs=3, space="PSUM"))
    ps_o = ctx.enter_context(tc.tile_pool(name="ps_o", bufs=1, space="PSUM"))

    # ---------------- constants / weights ----------------
    ident = consts.tile([128, 128], FP32)
    make_identity(nc, ident[:])

    w1t = consts.tile([63, H], FP32)               # W1^T  [63f, 256m]
    nc.sync.dma_start(out=w1t[:], in_=i_W1.rearrange("m f -> f m"))
    b1t = consts.tile([128, 2], FP32)
    nc.sync.dma_start(out=b1t[:], in_=i_b1.rearrange("(c p) -> p c", p=128))

    # Wih^T restricted to gates i,g,o: [128k, 2kc, 768m]; m-blocks: i=0:256, g=256:512, o=512:768
    wihT = []
    basum = []
    for lyr, (i_Wih, i_bih, i_bhh) in enumerate(
            ((i_Wih0, i_bih0, i_bhh0), (i_Wih1, i_bih1, i_bhh1))):
        wt = consts.tile([128, 2, 768], FP32, tag=f"wihT{lyr}")
        for gi, rs in enumerate((0, 2 * H, 3 * H)):   # i, g, o row offsets in 4H
            for c in range(2):
                nc.sync.dma_start(
                    out=wt[:, c, gi * H:(gi + 1) * H],
                    in_=i_Wih[rs:rs + H, c * 128:(c + 1) * 128].rearrange("m p -> p m"))
        wihT.append(wt)
        bi = consts.tile([128, 8], FP32, tag=f"btmp_i{lyr}")
        bh = consts.tile([128, 8], FP32, tag=f"btmp_h{lyr}")
        nc.sync.dma_start(out=bi[:], in_=i_bih.rearrange("(c p) -> p c", p=128))
        nc.sync.dma_start(out=bh[:], in_=i_bhh.rearrange("(c p) -> p c", p=128))
        bs = consts.tile([128, 8], FP32, tag=f"basum{lyr}")
        nc.vector.tensor_tensor(out=bs[:], in0=bi[:], in1=bh[:], op=A.add)
        basum.append(bs)
    # column in bs for gate-block m (0..5) = original 4H chunk index
    BIAS_COL = (0, 1, 4, 5, 6, 7)
    GATE_FUNC = (F.Sigmoid, F.Sigmoid, F.Tanh, F.Tanh, F.Sigmoid, F.Sigmoid)

    w2t = consts.tile([128, 2, 10], FP32)          # W2^T [256f -> 2x128, 10]
    for c in range(2):
        nc.sync.dma_start(out=w2t[:, c, :],
                          in_=i_W2[:, c * 128:(c + 1) * 128].rearrange("r p -> p r"))
    b2t = consts.tile([128, 10], FP32)
    nc.sync.dma_start(out=b2t[:], in_=bass.AP(tensor=i_b2.tensor, offset=i_b2.offset,
                                              ap=[[0, 128], [1, 10]]))
    delta_t = consts.tile([128, 1], FP32)
    nc.sync.dma_start(out=delta_t[:], in_=bass.AP(tensor=i_delta.tensor, offset=i_delta.offset,
                                                  ap=[[0, 128], [1, 1]]))
    pihalf = consts.tile([128, 1], FP32)
    nc.vector.memset(pihalf[:], float(np.pi / 2))

    # w = clip(weight + delta, 0, 1), batch-major [128, 32]
    w_raw = consts.tile([128, T_ALL], FP32)
    nc.sync.dma_start(out=w_raw[:], in_=i_w.rearrange("(t p) -> p t", p=128))
    w_t = consts.tile([128, T_ALL], FP32)
    nc.vector.tensor_scalar(out=w_t[:], in0=w_raw[:], scalar1=delta_t[:, 0:1],
                            scalar2=1.0, op0=A.add, op1=A.min)
    nc.vector.tensor_scalar(out=w_t[:], in0=w_t[:], scalar1=0.0, scalar2=None, op0=A.max)

    # out_bm accumulates the 10 MLP outputs, batch-major [128, 32t, 10]
    out_bm = consts.tile([128, T_ALL, 10], FP32)

    # per-supertile input/data tiles (created per s in loop, pooled)
    def prep_supertile(s):
        """Load inputs for supertile s and build data_bm [128, TS, 64] (batch-major)."""
        aM_s = stp.tile([128, TS, 6, 3], FP32, tag="aM_s")
        wM_s = stp.tile([128, TS, 6, 3], FP32, tag="wM_s")
        RMB_s = stp.tile([128, TS, 6, 3, 3], FP32, tag="RMB_s")
        r0 = s * TS * 128
        r1 = (s + 1) * TS * 128
        nc.sync.dma_start(out=aM_s[:], in_=i_aM[r0:r1].rearrange("(t p) n j -> p t n j", p=128))
        nc.sync.dma_start(out=wM_s[:], in_=i_wM[r0:r1].rearrange("(t p) n j -> p t n j", p=128))
        nc.sync.dma_start(out=RMB_s[:], in_=i_RMB[r0:r1].rearrange("(t p) n j k -> p t n j k", p=128))

        data_bm = stp.tile([128, TS, 64], FP32, tag="data_bm")
        # pad column 63 <- 0 (transposed into data_fm row 63, unused by W1 but keep clean)
        nc.vector.memset(bass.AP(tensor=data_bm.tensor, offset=data_bm.offset + 63,
                                 ap=[data_bm.ap[0], [64, TS], [1, 1]]), 0.0)
        # gR = -10 * R6[1, :]  -> cols 0:3
        nc.vector.tensor_scalar(out=data_bm[:, :, 0:3], in0=RMB_s[:, :, 5, 1, :],
                                scalar1=-10.0, scalar2=None, op0=A.mult)
        # d[n,j] = aM[n,j] - aM[5,j]
        d_t = stp.tile([128, TS, 5, 3], FP32, tag="d_t")
        nc.vector.tensor_tensor(out=d_t[:], in0=aM_s[:, :, 0:5, :],
                                in1=aM_s[:, :, 5:6, :].to_broadcast((128, TS, 5, 3)),
                                op=A.subtract)
        # acc[n,k] = sum_j d[n,j]*R6[j,k]  -> cols 3 + n*3 + k
        prod = stp.tile([128, TS, 5, 3], FP32, tag="prod")
        for k in range(3):
            r6jk = RMB_s[:, :, 5, :, k].unsqueeze(2).to_broadcast((128, TS, 5, 3))
            nc.vector.tensor_tensor(out=prod[:], in0=d_t[:], in1=r6jk, op=A.mult)
            acc_k = bass.AP(tensor=data_bm.tensor, offset=data_bm.offset + 3 + k,
                            ap=[data_bm.ap[0], [64, TS], [3, 5]])
            nc.vector.tensor_reduce(out=acc_k, in_=prod[:], axis=mybir.AxisListType.X, op=A.add)
        # ori[n,i,k] = sum_j R6[j,i]*RMB[n,j,k] -> cols 18 + n*9 + i*3 + k
        tmp = stp.tile([128, TS, 5, 3], FP32, tag="oritmp")
        for i in range(3):
            ori_i = bass.AP(tensor=data_bm.tensor, offset=data_bm.offset + 18 + i * 3,
                            ap=[data_bm.ap[0], [64, TS], [9, 5], [1, 3]])
            for j in range(3):
                r6ji = RMB_s[:, :, 5, j, i].unsqueeze(2).unsqueeze(3).to_broadcast((128, TS, 5, 3))
                if j == 0:
                    nc.vector.tensor_tensor(out=ori_i, in0=RMB_s[:, :, 0:5, j, :], in1=r6ji, op=A.mult)
                else:
                    nc.vector.tensor_tensor(out=tmp[:], in0=RMB_s[:, :, 0:5, j, :], in1=r6ji, op=A.mult)
                    nc.vector.tensor_tensor(out=ori_i, in0=ori_i, in1=tmp[:], op=A.add)
        return aM_s, wM_s, RMB_s, data_bm

    def lstm_batchtile(data_bm_tiles, bt):
        """Full MLP/LSTM chain for batch tile bt (512 elems); writes out_bm slice."""
        s = bt // (NBT // ST)
        data_bm = data_bm_tiles[s]
        t_base = bt * CPB - s * TS   # chunk offset inside supertile

        # transpose 4 chunks -> data_fm [64, 512]
        data_fm = btp.tile([64, NB], FP32, tag="data_fm")
        for t in range(CPB):
            pt = ps_tp.tile([64, 128], FP32, tag="tp")
            nc.tensor.transpose(pt[:], data_bm[:, t_base + t, :], ident[:])
            nc.vector.tensor_copy(out=data_fm[:, t * 128:(t + 1) * 128], in_=pt[:])

        # x0 = relu(W1 @ data + b1): [128, 2, 512]
        x0 = btp.tile([128, 2, NB], FP32, tag="x0")
        for m in range(2):
            px = ps_x0.tile([128, NB], FP32, tag="x0ps")
            nc.tensor.matmul(px[:], w1t[:, m * 128:(m + 1) * 128], data_fm[0:63, :],
                             start=True, stop=True)
            nc.scalar.activation(out=x0[:, m, :], in_=px[:], func=F.Relu,
                                 bias=b1t[:, m:m + 1], scale=1.0)

        # 2 LSTM layers
        h = x0
        for layer in range(2):
            wt = wihT[layer]
            bs = basum[layer]
            sig_i = btp.tile([128, 2, NB], FP32, tag=f"sig_i{layer}")
            tg = btp.tile([128, 2, NB], FP32, tag=f"tg{layer}")
            sig_o = btp.tile([128, 2, NB], FP32, tag=f"sig_o{layer}")
            dst = (sig_i, sig_i, tg, tg, sig_o, sig_o)
            for m in range(6):
                pg = ps_g.tile([128, NB], FP32, tag="gps")
                for k in range(2):
                    nc.tensor.matmul(pg[:], wt[:, k, m * 128:(m + 1) * 128], h[:, k, :],
                                     start=(k == 0), stop=(k == 1))
                nc.scalar.activation(out=dst[m][:, m % 2, :], in_=pg[:], func=GATE_FUNC[m],
                                     bias=bs[:, BIAS_COL[m]:BIAS_COL[m] + 1], scale=1.0)
            c2 = btp.tile([128, 2, NB], FP32, tag=f"c2_{layer}")
            nc.vector.tensor_tensor(out=c2[:], in0=sig_i[:], in1=tg[:], op=A.mult)
            tc2 = btp.tile([128, 2, NB], FP32, tag=f"tc2_{layer}")
            nc.scalar.activation(out=tc2[:], in_=c2[:], func=F.Tanh)
            hn = btp.tile([128, 2, NB], FP32, tag=f"h{layer}")
            nc.vector.tensor_tensor(out=hn[:], in0=sig_o[:], in1=tc2[:], op=A.mult)
            h = hn

        # out[b, 10] = h2^T @ W2^T  (flipped matmul -> batch-major)
        po = ps_o.tile([128, CPB, 10], FP32, tag="ops")
        for t in range(CPB):
            for k in range(2):
                nc.tensor.matmul(po[:, t, :], h[:, k, t * 128:(t + 1) * 128], w2t[:, k, :],
                                 start=(k == 0), stop=(k == 1))
        nc.vector.tensor_copy(out=out_bm[:, bt * CPB:(bt + 1) * CPB, :], in_=po[:])

    def angle_supertile(s):
        """atan2 + w-scale + sin/cos for supertile s -> c6/s6 [128, TS, 6]."""
        ob = out_bm[:, s * TS:(s + 1) * TS, :]
        sh5 = (128, TS, 5)
        # + b2
        nc.vector.tensor_tensor(out=ob, in0=ob,
                                in1=b2t[:].unsqueeze(1).to_broadcast((128, TS, 10)), op=A.add)
        axy = angp.tile([128, TS, 10], FP32, tag="axy")
        nc.scalar.activation(out=axy[:], in_=ob, func=F.Abs)
        ax, ay = axy[:, :, 0:5], axy[:, :, 5:10]
        x, y = ob[:, :, 0:5], ob[:, :, 5:10]
        mx = angp.tile([128, TS, 5], FP32, tag="mx")
        nc.vector.tensor_tensor(out=mx[:], in0=ax, in1=ay, op=A.max)
        nc.vector.tensor_scalar(out=mx[:], in0=mx[:], scalar1=1e-20, scalar2=None, op0=A.max)
        mn = angp.tile([128, TS, 5], FP32, tag="mn")
        nc.vector.tensor_tensor(out=mn[:], in0=ax, in1=ay, op=A.min)
        rinv = angp.tile([128, TS, 5], FP32, tag="rinv")
        nc.vector.reciprocal(out=rinv[:], in_=mx[:])
        q = angp.tile([128, TS, 5], FP32, tag="q")
        nc.vector.tensor_tensor(out=q[:], in0=mn[:], in1=rinv[:], op=A.mult)
        t0 = angp.tile([128, TS, 5], FP32, tag="t0")
        nc.scalar.activation(out=t0[:], in_=q[:], func=F.Arctan)
        # phi = t0 + (ay>ax)*(pi/2 - 2*t0)
        swp = angp.tile([128, TS, 5], FP32, tag="swp")
        nc.vector.tensor_tensor(out=swp[:], in0=ay, in1=ax, op=A.is_gt)
        tmp = angp.tile([128, TS, 5], FP32, tag="atmp")
        nc.vector.tensor_scalar(out=tmp[:], in0=t0[:], scalar1=-2.0, scalar2=float(np.pi / 2),
                                op0=A.mult, op1=A.add)
        nc.vector.tensor_tensor(out=tmp[:], in0=tmp[:], in1=swp[:], op=A.mult)
        phi = angp.tile([128, TS, 5], FP32, tag="phi")
        nc.vector.tensor_tensor(out=phi[:], in0=t0[:], in1=tmp[:], op=A.add)
        # phi = phi + (x<0)*(pi - 2*phi)
        xneg = angp.tile([128, TS, 5], FP32, tag="xneg")
        nc.vector.tensor_scalar(out=xneg[:], in0=x, scalar1=0.0, scalar2=None, op0=A.is_lt)
        nc.vector.tensor_scalar(out=tmp[:], in0=phi[:], scalar1=-2.0, scalar2=float(np.pi),
                                op0=A.mult, op1=A.add)
        nc.vector.tensor_tensor(out=tmp[:], in0=tmp[:], in1=xneg[:], op=A.mult)
        nc.vector.tensor_tensor(out=phi[:], in0=phi[:], in1=tmp[:], op=A.add)
        # angle = phi * sign(y);  phw = angle * w
        sgn = angp.tile([128, TS, 5], FP32, tag="sgn")
        nc.vector.tensor_scalar(out=sgn[:], in0=y, scalar1=0.0, scalar2=None, op0=A.is_lt)
        nc.vector.tensor_scalar(out=sgn[:], in0=sgn[:], scalar1=-2.0, scalar2=1.0,
                                op0=A.mult, op1=A.add)
        phw = angp.tile([128, TS, 5], FP32, tag="phw")
        nc.vector.tensor_tensor(out=phw[:], in0=phi[:], in1=sgn[:], op=A.mult)
        nc.vector.tensor_tensor(out=phw[:], in0=phw[:],
                                in1=w_t[:, s * TS:(s + 1) * TS].unsqueeze(2).to_broadcast(sh5),
                                op=A.mult)
        # s5 = sin(phw); c5 = sin(pi/2 - |phw|) = cos(phw)
        c6 = angp.tile([128, TS, 6], FP32, tag="c6")
        s6 = angp.tile([128, TS, 6], FP32, tag="s6")
        nc.scalar.activation(out=s6[:, :, 0:5], in_=phw[:], func=F.Sin)
        aphw = angp.tile([128, TS, 5], FP32, tag="aphw")
        nc.scalar.activation(out=aphw[:], in_=phw[:], func=F.Abs)
        nc.scalar.activation(out=c6[:, :, 0:5], in_=aphw[:], func=F.Sin,
                             bias=pihalf[:, 0:1], scale=-1.0)
        nc.vector.memset(c6[:, :, 5:6], 1.0)
        nc.vector.memset(s6[:, :, 5:6], 0.0)
        return c6, s6

    def rotate_supertile(s, aM_s, wM_s, RMB_s, c6, s6):
        """Apply y-rotation with cos=c6, sin_th=-s6 (th=-phw); DMA outputs."""
        r0 = s * TS * 128
        r1 = (s + 1) * TS * 128
        # vectors aM, wM: out0 = c*v0 - s*v2 ; out1 = v1 ; out2 = s*v0 + c*v2
        for src, dst_d, tag in ((aM_s, o_aMc, "a"), (wM_s, o_wMc, "w")):
            out_v = angp.tile([128, TS, 6, 3], FP32, tag=f"rot{tag}")
            t1 = angp.tile([128, TS, 6], FP32, tag="rv1")
            t2 = angp.tile([128, TS, 6], FP32, tag="rv2")
            v0, v1, v2 = src[:, :, :, 0], src[:, :, :, 1], src[:, :, :, 2]
            nc.vector.tensor_tensor(out=t1[:], in0=c6[:], in1=v0, op=A.mult)
            nc.vector.tensor_tensor(out=t2[:], in0=s6[:], in1=v2, op=A.mult)
            nc.vector.tensor_tensor(out=out_v[:, :, :, 0], in0=t1[:], in1=t2[:], op=A.subtract)
            nc.vector.tensor_copy(out=out_v[:, :, :, 1], in_=v1)
            nc.vector.tensor_tensor(out=t1[:], in0=s6[:], in1=v0, op=A.mult)
            nc.vector.tensor_tensor(out=t2[:], in0=c6[:], in1=v2, op=A.mult)
            nc.vector.tensor_tensor(out=out_v[:, :, :, 2], in0=t1[:], in1=t2[:], op=A.add)
            nc.sync.dma_start(out=dst_d[r0:r1].rearrange("(t p) n j -> p t n j", p=128),
                              in_=out_v[:])
        # matrices RMB: row_i transforms, each row is [3] over k
        sh = (128, TS, 6, 3)
        c6b = c6[:].unsqueeze(3).to_broadcast(sh)
        s6b = s6[:].unsqueeze(3).to_broadcast(sh)
        out_m = angp.tile([128, TS, 6, 3, 3], FP32, tag="rotR")
        m1 = angp.tile([128, TS, 6, 3], FP32, tag="rm1")
        m2 = angp.tile([128, TS, 6, 3], FP32, tag="rm2")
        R0, R1, R2 = RMB_s[:, :, :, 0, :], RMB_s[:, :, :, 1, :], RMB_s[:, :, :, 2, :]
        nc.vector.tensor_tensor(out=m1[:], in0=c6b, in1=R0, op=A.mult)
        nc.vector.tensor_tensor(out=m2[:], in0=s6b, in1=R2, op=A.mult)
        nc.vector.tensor_tensor(out=out_m[:, :, :, 0, :], in0=m1[:], in1=m2[:], op=A.subtract)
        nc.vector.tensor_copy(out=out_m[:, :, :, 1, :], in_=R1)
        nc.vector.tensor_tensor(out=m1[:], in0=s6b, in1=R0, op=A.mult)
        nc.vector.tensor_tensor(out=m2[:], in0=c6b, in1=R2, op=A.mult)
        nc.vector.tensor_tensor(out=out_m[:, :, :, 2, :], in0=m1[:], in1=m2[:], op=A.add)
        nc.sync.dma_start(out=o_RMBc[r0:r1].rearrange("(t p) n j k -> p t n j k", p=128),
                          in_=out_m[:])

    # ---------------- main schedule ----------------
    st_tiles = {}
    data_tiles = {}
    for s in range(ST):
        aM_s, wM_s, RMB_s, data_bm = prep_supertile(s)
        st_tiles[s] = (aM_s, wM_s, RMB_s)
        data_tiles[s] = data_bm
        for bt in range(s * (NBT // ST), (s + 1) * (NBT // ST)):
            lstm_batchtile(data_tiles, bt)
        c6, s6 = angle_supertile(s)
        rotate_supertile(s, *st_tiles[s], c6, s6)


def _get_module():
    if 'nc' not in _CACHE:
        _CACHE['nc'] = _build()
    return _CACHE['nc']


def kernel(**inputs):
    inp = {k: np.asarray(v) for k, v in inputs.items()}
    use_flag = int(np.asarray(inp.get('use_flag', 1)))
    delta = np.float32(1.0 / 90.0) if use_flag else np.float32(-1.0 / 90.0)

    nc = _get_module()

    rep = {}
    for name in ('W1', 'b1', 'Wih0', 'bih0', 'bhh0', 'Wih1', 'bih1', 'bhh1', 'W2', 'b2'):
        rep[name] = np.ascontiguousarray(inp[name], dtype=np.float32)
    rep['delta'] = np.array([delta], dtype=np.float32)

    in_maps = []
    for c in range(N_CORES):
        sl = slice(c * BC, (c + 1) * BC)
        m = dict(rep)
        m['aM'] = np.ascontiguousarray(inp['aM'][sl], dtype=np.float32)
        m['wM'] = np.ascontiguousarray(inp['wM'][sl], dtype=np.float32)
        m['RMB'] = np.ascontiguousarray(inp['RMB'][sl], dtype=np.float32)
        m['weight'] = np.ascontiguousarray(inp['weight'][sl], dtype=np.float32)
        in_maps.append(m)

    res = run_bass_kernel_spmd(nc, in_maps, core_ids=list(range(N_CORES)))
    kernel._last_result = res

    aMc = np.concatenate([res.results[c]['aMc'] for c in range(N_CORES)], axis=0)
    wMc = np.concatenate([res.results[c]['wMc'] for c in range(N_CORES)], axis=0)
    RMBc = np.concatenate([res.results[c]['RMBc'] for c in range(N_CORES)], axis=0)
    return aMc, wMc, RMBc


# revision 5
# speedup vs baseline: 1.0328x; 1.0328x over previous
"""Trainium2 Bass kernel for nn_CorrectorV2 (B=32768, H=256, 8 cores, data parallel).

Pipeline per core (Bc=4096 elements, batch-major <-> feature-major hybrid):
  1. preprocess (batch-major, DVE): data[b,63] = [gR, acc, ori] from aM/RMB
  2. PE-transpose data -> feature-major [63, n]
  3. x0 = relu(W1 @ data + b1)            (fp32 matmul, ACT relu+bias)
  4. 2x LSTM cell with zero init state -> only i,g,o gates needed (f is dead):
       h = sigmoid(o) * tanh(sigmoid(i) * tanh(g)),  g_pre = Wih x + bih + bhh
  5. out[b,10] = h2 @ W2.T + b2 (flipped matmul -> batch-major output)
  6. angle = atan2(sin_r, cos_r) via octant-reduced arctan; phi = angle*clip(weight+delta)
  7. rotate aM/wM/RMB about y by -phi (batch-major, DVE)

All matmuls are true fp32 (atan2 amplifies error by 1/|out| with |out|_min ~3e-4,
so reduced-precision matmuls are catastrophically wrong).
"""

import sys

if '/opt/trn_rl_repo' not in sys.path:
    sys.path.insert(0, '/opt/trn_rl_repo')

import numpy as np

import concourse.bass as bass
import concourse.bacc as bacc
import concourse.tile as tile
from concourse import mybir
from concourse.bass_utils import run_bass_kernel_spmd
from concourse.masks import make_identity
from concourse._compat import with_exitstack
from contextlib import ExitStack

F = mybir.ActivationFunctionType
A = mybir.AluOpType
FP32 = mybir.dt.float32

B = 32768
H = 256
N_CORES = 8
BC = B // N_CORES          # 4096 elements per core
T_ALL = BC // 128          # 32 chunks of 128
ST = 2                     # supertiles per core
TS = T_ALL // ST           # 16 chunks per supertile
NB = 512                   # LSTM batch tile
NBT = BC // NB             # 8 batch tiles
CPB = NB // 128            # 4 chunks per batch tile

_CACHE = {}


def _build():
    nc = bacc.Bacc("TRN2", target_bir_lowering=False, debug=False)

    # ---------------- DRAM tensors ----------------
    i_aM = nc.dram_tensor("aM", [BC, 6, 3], FP32, kind="ExternalInput").ap()
    i_wM = nc.dram_tensor("wM", [BC, 6, 3], FP32, kind="ExternalInput").ap()
    i_RMB = nc.dram_tensor("RMB", [BC, 6, 3, 3], FP32, kind="ExternalInput").ap()
    i_w = nc.dram_tensor("weight", [BC], FP32, kind="ExternalInput").ap()
    i_delta = nc.dram_tensor("delta", [1], FP32, kind="ExternalInput").ap()
    i_W1 = nc.dram_tensor("W1", [H, 63], FP32, kind="ExternalInput").ap()
    i_b1 = nc.dram_tensor("b1", [H], FP32, kind="ExternalInput").ap()
    i_Wih0 = nc.dram_tensor("Wih0", [4 * H, H], FP32, kind="ExternalInput").ap()
    i_bih0 = nc.dram_tensor("bih0", [4 * H], FP32, kind="ExternalInput").ap()
    i_bhh0 = nc.dram_tensor("bhh0", [4 * H], FP32, kind="ExternalInput").ap()
    i_Wih1 = nc.dram_tensor("Wih1", [4 * H, H], FP32, kind="ExternalInput").ap()
    i_bih1 = nc.dram_tensor("bih1", [4 * H], FP32, kind="ExternalInput").ap()
    i_bhh1 = nc.dram_tensor("bhh1", [4 * H], FP32, kind="ExternalInput").ap()
    i_W2 = nc.dram_tensor("W2", [10, H], FP32, kind="ExternalInput").ap()
    i_b2 = nc.dram_tensor("b2", [10], FP32, kind="ExternalInput").ap()
    o_aMc = nc.dram_tensor("aMc", [BC, 6, 3], FP32, kind="ExternalOutput").ap()
    o_wMc = nc.dram_tensor("wMc", [BC, 6, 3], FP32, kind="ExternalOutput").ap()
    o_RMBc = nc.dram_tensor("RMBc", [BC, 6, 3, 3], FP32, kind="ExternalOutput").ap()

    with tile.TileContext(nc) as tc:
        _emit(tc, i_aM, i_wM, i_RMB, i_w, i_delta, i_W1, i_b1,
              i_Wih0, i_bih0, i_bhh0, i_Wih1, i_bih1, i_bhh1, i_W2, i_b2,
              o_aMc, o_wMc, o_RMBc)
    nc.compile()
    return nc


@with_exitstack
def _emit(ctx: ExitStack, tc, i_aM, i_wM, i_RMB, i_w, i_delta, i_W1, i_b1,
          i_Wih0, i_bih0, i_bhh0, i_Wih1, i_bih1, i_bhh1, i_W2, i_b2,
          o_aMc, o_wMc, o_RMBc):
    nc = tc.nc
    consts = ctx.enter_context(tc.tile_pool(name="consts", bufs=1))
    stp = ctx.enter_context(tc.tile_pool(name="stp", bufs=2))       # supertile tiles
    btp = ctx.enter_context(tc.tile_pool(name="btp", bufs=2))       # batch-tile tiles
    angp = ctx.enter_context(tc.tile_pool(name="angp", bufs=2))     # angle/rot temps
    ps_tp = ctx.enter_context(tc.tile_pool(name="ps_tp", bufs=2, space="PSUM"))
    ps_x0 = ctx.enter_context(tc.tile_pool(name="ps_x0", bufs=2, space="PSUM"))
    ps_g = ctx.enter_context(tc.tile_pool(name="ps_g", bufs=3, space="PSUM"))
    ps_o = ctx.enter_context(tc.tile_pool(name="ps_o", bufs=1, space="PSUM"))

    # ---------------- constants / weights ----------------
    ident = consts.tile([128, 128], FP32)
    make_identity(nc, ident[:])

    w1t = consts.tile([63, H], FP32)               # W1^T  [63f, 256m]
    nc.sync.dma_start(out=w1t[:], in_=i_W1.rearrange("m f -> f m"))
    b1t = consts.tile([128, 2], FP32)
    nc.sync.dma_start(out=b1t[:], in_=i_b1.rearrange("(c p) -> p c", p=128))

    # Wih^T restricted to gates i,g,o: [128k, 2kc, 768m]; m-blocks: i=0:256, g=256:512, o=512:768
    wihT = []
    basum = []
    for lyr, (i_Wih, i_bih, i_bhh) in enumerate(
            ((i_Wih0, i_bih0, i_bhh0), (i_Wih1, i_bih1, i_bhh1))):
        wt = consts.tile([128, 2, 768], FP32, tag=f"wihT{lyr}")
        for gi, rs in enumerate((0, 2 * H, 3 * H)):   # i, g, o row offsets in 4H
            for c in range(2):
                nc.sync.dma_start(
                    out=wt[:, c, gi * H:(gi + 1) * H],
                    in_=i_Wih[rs:rs + H, c * 128:(c + 1) * 128].rearrange("m p -> p m"))
        wihT.append(wt)
        bi = consts.tile([128, 8], FP32, tag=f"btmp_i{lyr}")
        bh = consts.tile([128, 8], FP32, tag=f"btmp_h{lyr}")
        nc.sync.dma_start(out=bi[:], in_=i_bih.rearrange("(c p) -> p c", p=128))
        nc.sync.dma_start(out=bh[:], in_=i_bhh.rearrange("(c p) -> p c", p=128))
        bs = consts.tile([128, 8], FP32, tag=f"basum{lyr}")
        nc.vector.tensor_tensor(out=bs[:], in0=bi[:], in1=bh[:], op=A.add)
        basum.append(bs)
    # column in bs for gate-block m (0..5) = original 4H chunk index
    BIAS_COL = (0, 1, 4, 5, 6, 7)
    GATE_FUNC = (F.Sigmoid, F.Sigmoid, F.Tanh, F.Tanh, F.Sigmoid, F.Sigmoid)

    w2t = consts.tile([128, 2, 10], FP32)          # W2^T [256f -> 2x128, 10]
    for c in range(2):
        nc.sync.dma_start(out=w2t[:, c, :],
                          in_=i_W2[:, c * 128:(c + 1) * 128].rearrange("r p -> p r"))
    b2t = consts.tile([128, 10], FP32)
    nc.sync.dma_start(out=b2t[:], in_=bass.AP(tensor=i_b2.tensor, offset=i_b2.offset,
                                              ap=[[0, 128], [1, 10]]))
    delta_t = consts.tile([128, 1], FP32)
    nc.sync.dma_start(out=delta_t[:], in_=bass.AP(tensor=i_delta.tensor, offset=i_delta.offset,
                                                  ap=[[0, 128], [1, 1]]))
    pihalf = consts.tile([128, 1], FP32)
    nc.vector.memset(pihalf[:], float(np.pi / 2))

    # w = clip(weight + delta, 0, 1), batch-major [128, 32]
    w_raw = consts.tile([128, T_ALL], FP32)
    nc.sync.dma_start(out=w_raw[:], in_=i_w.rearrange("(p t) -> p t", p=128))
    w_t = consts.tile([128, T_ALL], FP32)
    nc.vector.tensor_scalar(out=w_t[:], in0=w_raw[:], scalar1=delta_t[:, 0:1],
                            scalar2=1.0, op0=A.add, op1=A.min)
    nc.vector.tensor_scalar(out=w_t[:], in0=w_t[:], scalar1=0.0, scalar2=None, op0=A.max)

    # out_bm accumulates the 10 MLP outputs, batch-major [128, 32t, 10]
    out_bm = consts.tile([128, T_ALL, 10], FP32)

    # per-supertile input/data tiles (created per s in loop, pooled)
    def prep_supertile(s):
        """Load inputs for supertile s and build data_bm [128, TS, 64] (batch-major)."""
        aM_s = stp.tile([128, TS, 6, 3], FP32, tag="aM_s")
        wM_s = stp.tile([128, TS, 6, 3], FP32, tag="wM_s")
        RMB_s = stp.tile([128, TS, 6, 3, 3], FP32, tag="RMB_s")
        sl = slice(s * TS, (s + 1) * TS)
        nc.sync.dma_start(out=aM_s[:],
                          in_=i_aM.rearrange("(p t) n j -> p t n j", p=128)[:, sl])
        nc.sync.dma_start(out=wM_s[:],
                          in_=i_wM.rearrange("(p t) n j -> p t n j", p=128)[:, sl])
        nc.sync.dma_start(out=RMB_s[:],
                          in_=i_RMB.rearrange("(p t) n j k -> p t n j k", p=128)[:, sl])

        data_bm = stp.tile([128, TS, 64], FP32, tag="data_bm")
        # pad column 63 <- 0 (transposed into data_fm row 63, unused by W1 but keep clean)
        nc.vector.memset(bass.AP(tensor=data_bm.tensor, offset=data_bm.offset + 63,
                                 ap=[data_bm.ap[0], [64, TS], [1, 1]]), 0.0)
        # gR = -10 * R6[1, :]  -> cols 0:3
        nc.vector.tensor_scalar(out=data_bm[:, :, 0:3], in0=RMB_s[:, :, 5, 1, :],
                                scalar1=-10.0, scalar2=None, op0=A.mult)
        # d[n,j] = aM[n,j] - aM[5,j]
        d_t = stp.tile([128, TS, 5, 3], FP32, tag="d_t")
        nc.vector.tensor_tensor(out=d_t[:], in0=aM_s[:, :, 0:5, :],
                                in1=aM_s[:, :, 5:6, :].to_broadcast((128, TS, 5, 3)),
                                op=A.subtract)
        # acc[n,k] = sum_j d[n,j]*R6[j,k]  -> cols 3 + n*3 + k
        prod = stp.tile([128, TS, 5, 3], FP32, tag="prod")
        for k in range(3):
            r6jk = RMB_s[:, :, 5, :, k].unsqueeze(2).to_broadcast((128, TS, 5, 3))
            nc.vector.tensor_tensor(out=prod[:], in0=d_t[:], in1=r6jk, op=A.mult)
            acc_k = bass.AP(tensor=data_bm.tensor, offset=data_bm.offset + 3 + k,
                            ap=[data_bm.ap[0], [64, TS], [3, 5]])
            nc.vector.tensor_reduce(out=acc_k, in_=prod[:], axis=mybir.AxisListType.X, op=A.add)
        # ori[n,i,k] = sum_j R6[j,i]*RMB[n,j,k] -> cols 18 + n*9 + i*3 + k
        tmp = stp.tile([128, TS, 5, 3], FP32, tag="oritmp")
        for i in range(3):
            ori_i = bass.AP(tensor=data_bm.tensor, offset=data_bm.offset + 18 + i * 3,
                            ap=[data_bm.ap[0], [64, TS], [9, 5], [1, 3]])
            for j in range(3):
                r6ji = RMB_s[:, :, 5, j, i].unsqueeze(2).unsqueeze(3).to_broadcast((128, TS, 5, 3))
                if j == 0:
                    nc.vector.tensor_tensor(out=ori_i, in0=RMB_s[:, :, 0:5, j, :], in1=r6ji, op=A.mult)
                else:
                    nc.vector.tensor_tensor(out=tmp[:], in0=RMB_s[:, :, 0:5, j, :], in1=r6ji, op=A.mult)
                    nc.vector.tensor_tensor(out=ori_i, in0=ori_i, in1=tmp[:], op=A.add)
        return aM_s, wM_s, RMB_s, data_bm

    def lstm_batchtile(data_bm_tiles, bt):
        """Full MLP/LSTM chain for batch tile bt (512 elems); writes out_bm slice."""
        s = bt // (NBT // ST)
        data_bm = data_bm_tiles[s]
        t_base = bt * CPB - s * TS   # chunk offset inside supertile

        # transpose 4 chunks -> data_fm [64, 512]
        data_fm = btp.tile([64, NB], FP32, tag="data_fm")
        for t in range(CPB):
            pt = ps_tp.tile([64, 128], FP32, tag="tp")
            nc.tensor.transpose(pt[:], data_bm[:, t_base + t, :], ident[:])
            nc.vector.tensor_copy(out=data_fm[:, t * 128:(t + 1) * 128], in_=pt[:])

        # x0 = relu(W1 @ data + b1): [128, 2, 512]
        x0 = btp.tile([128, 2, NB], FP32, tag="x0")
        for m in range(2):
            px = ps_x0.tile([128, NB], FP32, tag="x0ps")
            nc.tensor.matmul(px[:], w1t[:, m * 128:(m + 1) * 128], data_fm[0:63, :],
                             start=True, stop=True)
            nc.scalar.activation(out=x0[:, m, :], in_=px[:], func=F.Relu,
                                 bias=b1t[:, m:m + 1], scale=1.0)

        # 2 LSTM layers
        h = x0
        for layer in range(2):
            wt = wihT[layer]
            bs = basum[layer]
            sig_i = btp.tile([128, 2, NB], FP32, tag=f"sig_i{layer}")
            tg = btp.tile([128, 2, NB], FP32, tag=f"tg{layer}")
            sig_o = btp.tile([128, 2, NB], FP32, tag=f"sig_o{layer}")
            dst = (sig_i, sig_i, tg, tg, sig_o, sig_o)
            for m in range(6):
                pg = ps_g.tile([128, NB], FP32, tag="gps")
                for k in range(2):
                    nc.tensor.matmul(pg[:], wt[:, k, m * 128:(m + 1) * 128], h[:, k, :],
                                     start=(k == 0), stop=(k == 1))
                nc.scalar.activation(out=dst[m][:, m % 2, :], in_=pg[:], func=GATE_FUNC[m],
                                     bias=bs[:, BIAS_COL[m]:BIAS_COL[m] + 1], scale=1.0)
            c2 = btp.tile([128, 2, NB], FP32, tag=f"c2_{layer}")
            nc.vector.tensor_tensor(out=c2[:], in0=sig_i[:], in1=tg[:], op=A.mult)
            tc2 = btp.tile([128, 2, NB], FP32, tag=f"tc2_{layer}")
            nc.scalar.activation(out=tc2[:], in_=c2[:], func=F.Tanh)
            hn = btp.tile([128, 2, NB], FP32, tag=f"h{layer}")
            nc.vector.tensor_tensor(out=hn[:], in0=sig_o[:], in1=tc2[:], op=A.mult)
            h = hn

        # out[b, 10] = h2^T @ W2^T  (flipped matmul -> batch-major)
        po = ps_o.tile([128, CPB, 10], FP32, tag="ops")
        for t in range(CPB):
            for k in range(2):
                nc.tensor.matmul(po[:, t, :], h[:, k, t * 128:(t + 1) * 128], w2t[:, k, :],
                                 start=(k == 0), stop=(k == 1))
        nc.vector.tensor_copy(out=out_bm[:, bt * CPB:(bt + 1) * CPB, :], in_=po[:])

    def angle_supertile(s):
        """atan2 + w-scale + sin/cos for supertile s -> c6/s6 [128, TS, 6]."""
        ob = out_bm[:, s * TS:(s + 1) * TS, :]
        sh5 = (128, TS, 5)
        # + b2
        nc.vector.tensor_tensor(out=ob, in0=ob,
                                in1=b2t[:].unsqueeze(1).to_broadcast((128, TS, 10)), op=A.add)
        axy = angp.tile([128, TS, 10], FP32, tag="axy")
        nc.scalar.activation(out=axy[:], in_=ob, func=F.Abs)
        ax, ay = axy[:, :, 0:5], axy[:, :, 5:10]
        x, y = ob[:, :, 0:5], ob[:, :, 5:10]
        mx = angp.tile([128, TS, 5], FP32, tag="mx")
        nc.vector.tensor_tensor(out=mx[:], in0=ax, in1=ay, op=A.max)
        nc.vector.tensor_scalar(out=mx[:], in0=mx[:], scalar1=1e-20, scalar2=None, op0=A.max)
        mn = angp.tile([128, TS, 5], FP32, tag="mn")
        nc.vector.tensor_tensor(out=mn[:], in0=ax, in1=ay, op=A.min)
        rinv = angp.tile([128, TS, 5], FP32, tag="rinv")
        nc.vector.reciprocal(out=rinv[:], in_=mx[:])
        q = angp.tile([128, TS, 5], FP32, tag="q")
        nc.vector.tensor_tensor(out=q[:], in0=mn[:], in1=rinv[:], op=A.mult)
        t0 = angp.tile([128, TS, 5], FP32, tag="t0")
        nc.scalar.activation(out=t0[:], in_=q[:], func=F.Arctan)
        # phi = t0 + (ay>ax)*(pi/2 - 2*t0)
        swp = angp.tile([128, TS, 5], FP32, tag="swp")
        nc.vector.tensor_tensor(out=swp[:], in0=ay, in1=ax, op=A.is_gt)
        tmp = angp.tile([128, TS, 5], FP32, tag="atmp")
        nc.vector.tensor_scalar(out=tmp[:], in0=t0[:], scalar1=-2.0, scalar2=float(np.pi / 2),
                                op0=A.mult, op1=A.add)
        nc.vector.tensor_tensor(out=tmp[:], in0=tmp[:], in1=swp[:], op=A.mult)
        phi = angp.tile([128, TS, 5], FP32, tag="phi")
        nc.vector.tensor_tensor(out=phi[:], in0=t0[:], in1=tmp[:], op=A.add)
        # phi = phi + (x<0)*(pi - 2*phi)
        xneg = angp.tile([128, TS, 5], FP32, tag="xneg")
        nc.vector.tensor_scalar(out=xneg[:], in0=x, scalar1=0.0, scalar2=None, op0=A.is_lt)
        nc.vector.tensor_scalar(out=tmp[:], in0=phi[:], scalar1=-2.0, scalar2=float(np.pi),
                                op0=A.mult, op1=A.add)
        nc.vector.tensor_tensor(out=tmp[:], in0=tmp[:], in1=xneg[:], op=A.mult)
        nc.vector.tensor_tensor(out=phi[:], in0=phi[:], in1=tmp[:], op=A.add)
        # angle = phi * sign(y);  phw = angle * w
        sgn = angp.tile([128, TS, 5], FP32, tag="sgn")
        nc.vector.tensor_scalar(out=sgn[:], in0=y, scalar1=0.0, scalar2=None, op0=A.is_lt)
        nc.vector.tensor_scalar(out=sgn[:], in0=sgn[:], scalar1=-2.0, scalar2=1.0,
                                op0=A.mult, op1=A.add)
        phw = angp.tile([128, TS, 5], FP32, tag="phw")
        nc.vector.tensor_tensor(out=phw[:], in0=phi[:], in1=sgn[:], op=A.mult)
        nc.vector.tensor_tensor(out=phw[:], in0=phw[:],
                                in1=w_t[:, s * TS:(s + 1) * TS].unsqueeze(2).to_broadcast(sh5),
                                op=A.mult)
        # s5 = sin(phw); c5 = sin(pi/2 - |phw|) = cos(phw)
        c6 = angp.tile([128, TS, 6], FP32, tag="c6")
        s6 = angp.tile([128, TS, 6], FP32, tag="s6")
        nc.scalar.activation(out=s6[:, :, 0:5], in_=phw[:], func=F.Sin)
        aphw = angp.tile([128, TS, 5], FP32, tag="aphw")
        nc.scalar.activation(out=aphw[:], in_=phw[:], func=F.Abs)
        nc.scalar.activation(out=c6[:, :, 0:5], in_=aphw[:], func=F.Sin,
                             bias=pihalf[:, 0:1], scale=-1.0)
        nc.vector.memset(c6[:, :, 5:6], 1.0)
        nc.vector.memset(s6[:, :, 5:6], 0.0)
        return c6, s6

    def rotate_supertile(s, aM_s, wM_s, RMB_s, c6, s6):
        """Apply y-rotation with cos=c6, sin_th=-s6 (th=-phw); DMA outputs."""
        sl = slice(s * TS, (s + 1) * TS)
        # vectors aM, wM: out0 = c*v0 - s*v2 ; out1 = v1 ; out2 = s*v0 + c*v2
        for src, dst_d, tag in ((aM_s, o_aMc, "a"), (wM_s, o_wMc, "w")):
            out_v = angp.tile([128, TS, 6, 3], FP32, tag=f"rot{tag}")
            t1 = angp.tile([128, TS, 6], FP32, tag="rv1")
            t2 = angp.tile([128, TS, 6], FP32, tag="rv2")
            v0, v1, v2 = src[:, :, :, 0], src[:, :, :, 1], src[:, :, :, 2]
            nc.vector.tensor_tensor(out=t1[:], in0=c6[:], in1=v0, op=A.mult)
            nc.vector.tensor_tensor(out=t2[:], in0=s6[:], in1=v2, op=A.mult)
            nc.vector.tensor_tensor(out=out_v[:, :, :, 0], in0=t1[:], in1=t2[:], op=A.subtract)
            nc.vector.tensor_copy(out=out_v[:, :, :, 1], in_=v1)
            nc.vector.tensor_tensor(out=t1[:], in0=s6[:], in1=v0, op=A.mult)
            nc.vector.tensor_tensor(out=t2[:], in0=c6[:], in1=v2, op=A.mult)
            nc.vector.tensor_tensor(out=out_v[:, :, :, 2], in0=t1[:], in1=t2[:], op=A.add)
            nc.sync.dma_start(out=dst_d.rearrange("(p t) n j -> p t n j", p=128)[:, sl],
                              in_=out_v[:])
        # matrices RMB: row_i transforms, each row is [3] over k
        sh = (128, TS, 6, 3)
        c6b = c6[:].unsqueeze(3).to_broadcast(sh)
        s6b = s6[:].unsqueeze(3).to_broadcast(sh)
        out_m = angp.tile([128, TS, 6, 3, 3], FP32, tag="rotR")
        m1 = angp.tile([128, TS, 6, 3], FP32, tag="rm1")
        m2 = angp.tile([128, TS, 6, 3], FP32, tag="rm2")
        R0, R1, R2 = RMB_s[:, :, :, 0, :], RMB_s[:, :, :, 1, :], RMB_s[:, :, :, 2, :]
        nc.vector.tensor_tensor(out=m1[:], in0=c6b, in1=R0, op=A.mult)
        nc.vector.tensor_tensor(out=m2[:], in0=s6b, in1=R2, op=A.mult)
        nc.vector.tensor_tensor(out=out_m[:, :, :, 0, :], in0=m1[:], in1=m2[:], op=A.subtract)
        nc.vector.tensor_copy(out=out_m[:, :, :, 1, :], in_=R1)
        nc.vector.tensor_tensor(out=m1[:], in0=s6b, in1=R0, op=A.mult)
        nc.vector.tensor_tensor(out=m2[:], in0=c6b, in1=R2, op=A.mult)
        nc.vector.tensor_tensor(out=out_m[:, :, :, 2, :], in0=m1[:], in1=m2[:], op=A.add)
        nc.sync.dma_start(out=o_RMBc.rearrange("(p t) n j k -> p t n j k", p=128)[:, sl],
                          in_=out_m[:])

    # ---------------- main schedule ----------------
    st_tiles = {}
    data_tiles = {}
    for s in range(ST):
        aM_s, wM_s, RMB_s, data_bm = prep_supertile(s)
        st_tiles[s] = (aM_s, wM_s, RMB_s)
        data_tiles[s] = data_bm
        for bt in range(s * (NBT // ST), (s + 1) * (NBT // ST)):
            lstm_batchtile(data_tiles, bt)
        c6, s6 = angle_supertile(s)
        rotate_supertile(s, *st_tiles[s], c6, s6)


def _get_module():
    if 'nc' not in _CACHE:
        _CACHE['nc'] = _build()
    return _CACHE['nc']


def kernel(**inputs):
    inp = {k: np.asarray(v) for k, v in inputs.items()}
    use_flag = int(np.asarray(inp.get('use_flag', 1)))
    delta = np.float32(1.0 / 90.0) if use_flag else np.float32(-1.0 / 90.0)

    nc = _get_module()

    rep = {}
    for name in ('W1', 'b1', 'Wih0', 'bih0', 'bhh0', 'Wih1', 'bih1', 'bhh1', 'W2', 'b2'):
        rep[name] = np.ascontiguousarray(inp[name], dtype=np.float32)
    rep['delta'] = np.array([delta], dtype=np.float32)

    in_maps = []
    for c in range(N_CORES):
        sl = slice(c * BC, (c + 1) * BC)
        m = dict(rep)
        m['aM'] = np.ascontiguousarray(inp['aM'][sl], dtype=np.float32)
        m['wM'] = np.ascontiguousarray(inp['wM'][sl], dtype=np.float32)
        m['RMB'] = np.ascontiguousarray(inp['RMB'][sl], dtype=np.float32)
        m['weight'] = np.ascontiguousarray(inp['weight'][sl], dtype=np.float32)
        in_maps.append(m)

    res = run_bass_kernel_spmd(nc, in_maps, core_ids=list(range(N_CORES)))
    kernel._last_result = res

    aMc = np.concatenate([res.results[c]['aMc'] for c in range(N_CORES)], axis=0)
    wMc = np.concatenate([res.results[c]['wMc'] for c in range(N_CORES)], axis=0)
    RMBc = np.concatenate([res.results[c]['RMBc'] for c in range(N_CORES)], axis=0)
    return aMc, wMc, RMBc


# revision 7
# speedup vs baseline: 1.8561x; 1.7972x over previous
"""Trainium2 Bass kernel for nn_CorrectorV2 (B=32768, H=256, 8 cores, data parallel).

Pipeline per core (Bc=4096 elements, batch-major <-> feature-major hybrid):
  1. preprocess (batch-major, DVE): data[b,63] = [gR, acc, ori] from aM/RMB
  2. PE-transpose data -> feature-major [63, n]
  3. x0 = relu(W1 @ data + b1)            (fp32 matmul, ACT relu+bias)
  4. 2x LSTM cell with zero init state -> only i,g,o gates needed (f is dead):
       h = sigmoid(o) * tanh(sigmoid(i) * tanh(g)),  g_pre = Wih x + bih + bhh
  5. out[b,10] = h2 @ W2.T + b2 (flipped matmul -> batch-major output)
  6. angle = atan2(sin_r, cos_r) via octant-reduced arctan; phi = angle*clip(weight+delta)
  7. rotate aM/wM/RMB about y by -phi (batch-major, DVE)

All matmuls are true fp32 (atan2 amplifies error by 1/|out| with |out|_min ~3e-4,
so reduced-precision matmuls are catastrophically wrong).
"""

import sys

if '/opt/trn_rl_repo' not in sys.path:
    sys.path.insert(0, '/opt/trn_rl_repo')

import numpy as np

import concourse.bass as bass
import concourse.bacc as bacc
import concourse.tile as tile
from concourse import mybir
from concourse.bass_utils import run_bass_kernel_spmd
from concourse.masks import make_identity
from concourse._compat import with_exitstack
from contextlib import ExitStack

F = mybir.ActivationFunctionType
A = mybir.AluOpType
FP32 = mybir.dt.float32

B = 32768
H = 256
N_CORES = 8
BC = B // N_CORES          # 4096 elements per core
T_ALL = BC // 128          # 32 chunks of 128
ST = 2                     # supertiles per core
TS = T_ALL // ST           # 16 chunks per supertile
NB = 512                   # LSTM batch tile
NBT = BC // NB             # 8 batch tiles
CPB = NB // 128            # 4 chunks per batch tile

_CACHE = {}


def _build():
    nc = bacc.Bacc("TRN2", target_bir_lowering=False, debug=False)

    # ---------------- DRAM tensors ----------------
    i_aM = nc.dram_tensor("aM", [BC, 6, 3], FP32, kind="ExternalInput").ap()
    i_wM = nc.dram_tensor("wM", [BC, 6, 3], FP32, kind="ExternalInput").ap()
    i_RMB = nc.dram_tensor("RMB", [BC, 6, 3, 3], FP32, kind="ExternalInput").ap()
    i_w = nc.dram_tensor("weight", [BC], FP32, kind="ExternalInput").ap()
    i_delta = nc.dram_tensor("delta", [1], FP32, kind="ExternalInput").ap()
    i_W1 = nc.dram_tensor("W1", [H, 63], FP32, kind="ExternalInput").ap()
    i_b1 = nc.dram_tensor("b1", [H], FP32, kind="ExternalInput").ap()
    i_Wih0 = nc.dram_tensor("Wih0", [4 * H, H], FP32, kind="ExternalInput").ap()
    i_bih0 = nc.dram_tensor("bih0", [4 * H], FP32, kind="ExternalInput").ap()
    i_bhh0 = nc.dram_tensor("bhh0", [4 * H], FP32, kind="ExternalInput").ap()
    i_Wih1 = nc.dram_tensor("Wih1", [4 * H, H], FP32, kind="ExternalInput").ap()
    i_bih1 = nc.dram_tensor("bih1", [4 * H], FP32, kind="ExternalInput").ap()
    i_bhh1 = nc.dram_tensor("bhh1", [4 * H], FP32, kind="ExternalInput").ap()
    i_W2 = nc.dram_tensor("W2", [10, H], FP32, kind="ExternalInput").ap()
    i_b2 = nc.dram_tensor("b2", [10], FP32, kind="ExternalInput").ap()
    o_aMc = nc.dram_tensor("aMc", [BC, 6, 3], FP32, kind="ExternalOutput").ap()
    o_wMc = nc.dram_tensor("wMc", [BC, 6, 3], FP32, kind="ExternalOutput").ap()
    o_RMBc = nc.dram_tensor("RMBc", [BC, 6, 3, 3], FP32, kind="ExternalOutput").ap()

    with tile.TileContext(nc) as tc:
        _emit(tc, i_aM, i_wM, i_RMB, i_w, i_delta, i_W1, i_b1,
              i_Wih0, i_bih0, i_bhh0, i_Wih1, i_bih1, i_bhh1, i_W2, i_b2,
              o_aMc, o_wMc, o_RMBc)
    nc.compile()
    return nc


@with_exitstack
def _emit(ctx: ExitStack, tc, i_aM, i_wM, i_RMB, i_w, i_delta, i_W1, i_b1,
          i_Wih0, i_bih0, i_bhh0, i_Wih1, i_bih1, i_bhh1, i_W2, i_b2,
          o_aMc, o_wMc, o_RMBc):
    nc = tc.nc
    consts = ctx.enter_context(tc.tile_pool(name="consts", bufs=1))
    stp = ctx.enter_context(tc.tile_pool(name="stp", bufs=2))       # supertile tiles
    btp = ctx.enter_context(tc.tile_pool(name="btp", bufs=2))       # batch-tile tiles
    angp = ctx.enter_context(tc.tile_pool(name="angp", bufs=2))     # angle/rot temps
    ps_tp = ctx.enter_context(tc.tile_pool(name="ps_tp", bufs=2, space="PSUM"))
    ps_x0 = ctx.enter_context(tc.tile_pool(name="ps_x0", bufs=2, space="PSUM"))
    ps_g = ctx.enter_context(tc.tile_pool(name="ps_g", bufs=3, space="PSUM"))
    ps_o = ctx.enter_context(tc.tile_pool(name="ps_o", bufs=1, space="PSUM"))

    # ---------------- constants / weights ----------------
    ident = consts.tile([128, 128], FP32)
    make_identity(nc, ident[:])

    w1t = consts.tile([63, H], FP32)               # W1^T  [63f, 256m]
    nc.sync.dma_start(out=w1t[:], in_=i_W1.rearrange("m f -> f m"))
    b1t = consts.tile([128, 2], FP32)
    b1r = consts.tile([2, 128], FP32, tag="b1r")
    nc.sync.dma_start(out=b1r[:], in_=i_b1.rearrange("(c p) -> c p", p=128))
    pw = ps_tp.tile([128, 128], FP32, tag="tp")
    nc.tensor.transpose(pw[:, 0:2], b1r[:], ident[0:2, 0:2])
    nc.vector.tensor_copy(out=b1t[:], in_=pw[:, 0:2])

    # Wih^T restricted to gates i,g,o: [128k, 2kc, 768m]; m-blocks: i=0:256, g=256:512, o=512:768
    wihT = []
    basum = []
    for lyr, (i_Wih, i_bih, i_bhh) in enumerate(
            ((i_Wih0, i_bih0, i_bhh0), (i_Wih1, i_bih1, i_bhh1))):
        wt = consts.tile([128, 2, 768], FP32, tag=f"wihT{lyr}")
        wtv = wt[:].rearrange("p c m -> p (c m)")
        for gi, rs in enumerate((0, 2 * H, 3 * H)):   # i, g, o row offsets in 4H
            # dst view [256 feat, 256 gate-rows] for this gate group:
            # wt[kp, c, gi*H + (rc*128 + j)] -> partition kp, col c*768 + gi*H + ...
            raw = consts.tile([128, 2, H], FP32, tag=f"wr{lyr}{gi}")
            nc.sync.dma_start(out=raw[:],
                              in_=i_Wih[rs:rs + H, :].rearrange("(c p) m -> p c m", p=128))
            for rc in range(2):       # gate-row half (m-block = gi*2 + rc)
                for kc in range(2):   # feature chunk
                    pw = ps_tp.tile([128, 128], FP32, tag="tp")
                    nc.tensor.transpose(pw[:], raw[:, rc, kc * 128:(kc + 1) * 128], ident[:])
                    nc.vector.tensor_copy(out=wt[:, kc, (gi * 2 + rc) * 128:(gi * 2 + rc + 1) * 128],
                                          in_=pw[:])
        wihT.append(wt)
        bi = consts.tile([8, 128], FP32, tag=f"btmp_i{lyr}")
        bh = consts.tile([8, 128], FP32, tag=f"btmp_h{lyr}")
        nc.sync.dma_start(out=bi[:], in_=i_bih.rearrange("(c p) -> c p", p=128))
        nc.sync.dma_start(out=bh[:], in_=i_bhh.rearrange("(c p) -> c p", p=128))
        br = consts.tile([8, 128], FP32, tag=f"braw{lyr}")
        nc.vector.tensor_tensor(out=br[:], in0=bi[:], in1=bh[:], op=A.add)
        bs = consts.tile([128, 8], FP32, tag=f"basum{lyr}")
        pw = ps_tp.tile([128, 128], FP32, tag="tp")
        nc.tensor.transpose(pw[:, 0:8], br[:], ident[0:8, 0:8])
        nc.vector.tensor_copy(out=bs[:], in_=pw[:, 0:8])
        basum.append(bs)
    # column in bs for gate-block m (0..5) = original 4H chunk index
    BIAS_COL = (0, 1, 4, 5, 6, 7)
    GATE_FUNC = (F.Sigmoid, F.Sigmoid, F.Tanh, F.Tanh, F.Sigmoid, F.Sigmoid)

    w2t = consts.tile([128, 2, 10], FP32)          # W2^T [256f -> 2x128, 10]
    for c in range(2):
        nc.sync.dma_start(out=w2t[:, c, :],
                          in_=i_W2[:, c * 128:(c + 1) * 128].rearrange("r p -> p r"))
    b2t = consts.tile([128, 10], FP32)
    nc.sync.dma_start(out=b2t[:], in_=bass.AP(tensor=i_b2.tensor, offset=i_b2.offset,
                                              ap=[[0, 128], [1, 10]]))
    delta_t = consts.tile([128, 1], FP32)
    nc.sync.dma_start(out=delta_t[:], in_=bass.AP(tensor=i_delta.tensor, offset=i_delta.offset,
                                                  ap=[[0, 128], [1, 1]]))
    pihalf = consts.tile([128, 1], FP32)
    nc.vector.memset(pihalf[:], float(np.pi / 2))

    # w = clip(weight + delta, 0, 1), batch-major [128, 32]
    w_raw = consts.tile([128, T_ALL], FP32)
    nc.sync.dma_start(out=w_raw[:], in_=i_w.rearrange("(p t) -> p t", p=128))
    w_t = consts.tile([128, T_ALL], FP32)
    nc.vector.tensor_scalar(out=w_t[:], in0=w_raw[:], scalar1=delta_t[:, 0:1],
                            scalar2=1.0, op0=A.add, op1=A.min)
    nc.vector.tensor_scalar(out=w_t[:], in0=w_t[:], scalar1=0.0, scalar2=None, op0=A.max)

    # out_bm accumulates the 10 MLP outputs, batch-major [128, 32t, 10]
    out_bm = consts.tile([128, T_ALL, 10], FP32)

    # per-supertile input/data tiles (created per s in loop, pooled)
    def prep_supertile(s):
        """Load inputs for supertile s and build data_bm [128, TS, 64] (batch-major)."""
        aM_s = stp.tile([128, TS, 6, 3], FP32, tag="aM_s")
        wM_s = stp.tile([128, TS, 6, 3], FP32, tag="wM_s")
        RMB_s = stp.tile([128, TS, 6, 3, 3], FP32, tag="RMB_s")
        sl = slice(s * TS, (s + 1) * TS)
        nc.sync.dma_start(out=aM_s[:],
                          in_=i_aM.rearrange("(p t) n j -> p t n j", p=128)[:, sl])
        nc.sync.dma_start(out=wM_s[:],
                          in_=i_wM.rearrange("(p t) n j -> p t n j", p=128)[:, sl])
        nc.sync.dma_start(out=RMB_s[:],
                          in_=i_RMB.rearrange("(p t) n j k -> p t n j k", p=128)[:, sl])

        data_bm = stp.tile([128, TS, 64], FP32, tag="data_bm")
        # pad column 63 <- 0 (transposed into data_fm row 63, unused by W1 but keep clean)
        nc.vector.memset(bass.AP(tensor=data_bm.tensor, offset=data_bm.offset + 63,
                                 ap=[data_bm.ap[0], [64, TS], [1, 1]]), 0.0)
        # gR = -10 * R6[1, :]  -> cols 0:3
        nc.vector.tensor_scalar(out=data_bm[:, :, 0:3], in0=RMB_s[:, :, 5, 1, :],
                                scalar1=-10.0, scalar2=None, op0=A.mult)
        # d[n,j] = aM[n,j] - aM[5,j]
        d_t = stp.tile([128, TS, 5, 3], FP32, tag="d_t")
        nc.vector.tensor_tensor(out=d_t[:], in0=aM_s[:, :, 0:5, :],
                                in1=aM_s[:, :, 5:6, :].to_broadcast((128, TS, 5, 3)),
                                op=A.subtract)
        # acc[n,k] = sum_j d[n,j]*R6[j,k]  -> cols 3 + n*3 + k
        prod = stp.tile([128, TS, 5, 3], FP32, tag="prod")
        for k in range(3):
            r6jk = RMB_s[:, :, 5, :, k].unsqueeze(2).to_broadcast((128, TS, 5, 3))
            nc.vector.tensor_tensor(out=prod[:], in0=d_t[:], in1=r6jk, op=A.mult)
            acc_k = bass.AP(tensor=data_bm.tensor, offset=data_bm.offset + 3 + k,
                            ap=[data_bm.ap[0], [64, TS], [3, 5]])
            nc.vector.tensor_reduce(out=acc_k, in_=prod[:], axis=mybir.AxisListType.X, op=A.add)
        # ori[n,i,k] = sum_j R6[j,i]*RMB[n,j,k] -> cols 18 + n*9 + i*3 + k
        tmp = stp.tile([128, TS, 5, 3], FP32, tag="oritmp")
        for i in range(3):
            ori_i = bass.AP(tensor=data_bm.tensor, offset=data_bm.offset + 18 + i * 3,
                            ap=[data_bm.ap[0], [64, TS], [9, 5], [1, 3]])
            for j in range(3):
                r6ji = RMB_s[:, :, 5, j, i].unsqueeze(2).unsqueeze(3).to_broadcast((128, TS, 5, 3))
                if j == 0:
                    nc.vector.tensor_tensor(out=ori_i, in0=RMB_s[:, :, 0:5, j, :], in1=r6ji, op=A.mult)
                else:
                    nc.vector.tensor_tensor(out=tmp[:], in0=RMB_s[:, :, 0:5, j, :], in1=r6ji, op=A.mult)
                    nc.vector.tensor_tensor(out=ori_i, in0=ori_i, in1=tmp[:], op=A.add)
        return aM_s, wM_s, RMB_s, data_bm

    def lstm_batchtile(data_bm_tiles, bt):
        """Full MLP/LSTM chain for batch tile bt (512 elems); writes out_bm slice."""
        s = bt // (NBT // ST)
        data_bm = data_bm_tiles[s]
        t_base = bt * CPB - s * TS   # chunk offset inside supertile

        # transpose 4 chunks -> data_fm [64, 512]
        data_fm = btp.tile([64, NB], FP32, tag="data_fm")
        for t in range(CPB):
            pt = ps_tp.tile([64, 128], FP32, tag="tp")
            nc.tensor.transpose(pt[:], data_bm[:, t_base + t, :], ident[:])
            nc.vector.tensor_copy(out=data_fm[:, t * 128:(t + 1) * 128], in_=pt[:])

        # x0 = relu(W1 @ data + b1): [128, 2, 512]
        x0 = btp.tile([128, 2, NB], FP32, tag="x0")
        for m in range(2):
            px = ps_x0.tile([128, NB], FP32, tag="x0ps")
            nc.tensor.matmul(px[:], w1t[:, m * 128:(m + 1) * 128], data_fm[0:63, :],
                             start=True, stop=True)
            nc.scalar.activation(out=x0[:, m, :], in_=px[:], func=F.Relu,
                                 bias=b1t[:, m:m + 1], scale=1.0)

        # 2 LSTM layers
        h = x0
        for layer in range(2):
            wt = wihT[layer]
            bs = basum[layer]
            sig_i = btp.tile([128, 2, NB], FP32, tag=f"sig_i{layer}")
            tg = btp.tile([128, 2, NB], FP32, tag=f"tg{layer}")
            sig_o = btp.tile([128, 2, NB], FP32, tag=f"sig_o{layer}")
            dst = (sig_i, sig_i, tg, tg, sig_o, sig_o)
            for m in range(6):
                pg = ps_g.tile([128, NB], FP32, tag="gps")
                for k in range(2):
                    nc.tensor.matmul(pg[:], wt[:, k, m * 128:(m + 1) * 128], h[:, k, :],
                                     start=(k == 0), stop=(k == 1))
                nc.scalar.activation(out=dst[m][:, m % 2, :], in_=pg[:], func=GATE_FUNC[m],
                                     bias=bs[:, BIAS_COL[m]:BIAS_COL[m] + 1], scale=1.0)
            c2 = btp.tile([128, 2, NB], FP32, tag=f"c2_{layer}")
            nc.vector.tensor_tensor(out=c2[:], in0=sig_i[:], in1=tg[:], op=A.mult)
            tc2 = btp.tile([128, 2, NB], FP32, tag=f"tc2_{layer}")
            nc.scalar.activation(out=tc2[:], in_=c2[:], func=F.Tanh)
            hn = btp.tile([128, 2, NB], FP32, tag=f"h{layer}")
            nc.vector.tensor_tensor(out=hn[:], in0=sig_o[:], in1=tc2[:], op=A.mult)
            h = hn

        # out[b, 10] = h2^T @ W2^T  (flipped matmul -> batch-major)
        po = ps_o.tile([128, CPB, 10], FP32, tag="ops")
        for t in range(CPB):
            for k in range(2):
                nc.tensor.matmul(po[:, t, :], h[:, k, t * 128:(t + 1) * 128], w2t[:, k, :],
                                 start=(k == 0), stop=(k == 1))
        nc.vector.tensor_copy(out=out_bm[:, bt * CPB:(bt + 1) * CPB, :], in_=po[:])

    def angle_supertile(s):
        """atan2 + w-scale + sin/cos for supertile s -> c6/s6 [128, TS, 6]."""
        ob = out_bm[:, s * TS:(s + 1) * TS, :]
        sh5 = (128, TS, 5)
        # + b2
        nc.vector.tensor_tensor(out=ob, in0=ob,
                                in1=b2t[:].unsqueeze(1).to_broadcast((128, TS, 10)), op=A.add)
        axy = angp.tile([128, TS, 10], FP32, tag="axy")
        nc.scalar.activation(out=axy[:], in_=ob, func=F.Abs)
        ax, ay = axy[:, :, 0:5], axy[:, :, 5:10]
        x, y = ob[:, :, 0:5], ob[:, :, 5:10]
        mx = angp.tile([128, TS, 5], FP32, tag="mx")
        nc.vector.tensor_tensor(out=mx[:], in0=ax, in1=ay, op=A.max)
        nc.vector.tensor_scalar(out=mx[:], in0=mx[:], scalar1=1e-20, scalar2=None, op0=A.max)
        mn = angp.tile([128, TS, 5], FP32, tag="mn")
        nc.vector.tensor_tensor(out=mn[:], in0=ax, in1=ay, op=A.min)
        rinv = angp.tile([128, TS, 5], FP32, tag="rinv")
        nc.vector.reciprocal(out=rinv[:], in_=mx[:])
        q = angp.tile([128, TS, 5], FP32, tag="q")
        nc.vector.tensor_tensor(out=q[:], in0=mn[:], in1=rinv[:], op=A.mult)
        t0 = angp.tile([128, TS, 5], FP32, tag="t0")
        nc.scalar.activation(out=t0[:], in_=q[:], func=F.Arctan)
        # phi = t0 + (ay>ax)*(pi/2 - 2*t0)
        swp = angp.tile([128, TS, 5], FP32, tag="swp")
        nc.vector.tensor_tensor(out=swp[:], in0=ay, in1=ax, op=A.is_gt)
        tmp = angp.tile([128, TS, 5], FP32, tag="atmp")
        nc.vector.tensor_scalar(out=tmp[:], in0=t0[:], scalar1=-2.0, scalar2=float(np.pi / 2),
                                op0=A.mult, op1=A.add)
        nc.vector.tensor_tensor(out=tmp[:], in0=tmp[:], in1=swp[:], op=A.mult)
        phi = angp.tile([128, TS, 5], FP32, tag="phi")
        nc.vector.tensor_tensor(out=phi[:], in0=t0[:], in1=tmp[:], op=A.add)
        # phi = phi + (x<0)*(pi - 2*phi)
        xneg = angp.tile([128, TS, 5], FP32, tag="xneg")
        nc.vector.tensor_scalar(out=xneg[:], in0=x, scalar1=0.0, scalar2=None, op0=A.is_lt)
        nc.vector.tensor_scalar(out=tmp[:], in0=phi[:], scalar1=-2.0, scalar2=float(np.pi),
                                op0=A.mult, op1=A.add)
        nc.vector.tensor_tensor(out=tmp[:], in0=tmp[:], in1=xneg[:], op=A.mult)
        nc.vector.tensor_tensor(out=phi[:], in0=phi[:], in1=tmp[:], op=A.add)
        # angle = phi * sign(y);  phw = angle * w
        sgn = angp.tile([128, TS, 5], FP32, tag="sgn")
        nc.vector.tensor_scalar(out=sgn[:], in0=y, scalar1=0.0, scalar2=None, op0=A.is_lt)
        nc.vector.tensor_scalar(out=sgn[:], in0=sgn[:], scalar1=-2.0, scalar2=1.0,
                                op0=A.mult, op1=A.add)
        phw = angp.tile([128, TS, 5], FP32, tag="phw")
        nc.vector.tensor_tensor(out=phw[:], in0=phi[:], in1=sgn[:], op=A.mult)
        nc.vector.tensor_tensor(out=phw[:], in0=phw[:],
                                in1=w_t[:, s * TS:(s + 1) * TS].unsqueeze(2).to_broadcast(sh5),
                                op=A.mult)
        # s5 = sin(phw); c5 = sin(pi/2 - |phw|) = cos(phw)
        c6 = angp.tile([128, TS, 6], FP32, tag="c6")
        s6 = angp.tile([128, TS, 6], FP32, tag="s6")
        nc.scalar.activation(out=s6[:, :, 0:5], in_=phw[:], func=F.Sin)
        aphw = angp.tile([128, TS, 5], FP32, tag="aphw")
        nc.scalar.activation(out=aphw[:], in_=phw[:], func=F.Abs)
        nc.scalar.activation(out=c6[:, :, 0:5], in_=aphw[:], func=F.Sin,
                             bias=pihalf[:, 0:1], scale=-1.0)
        nc.vector.memset(c6[:, :, 5:6], 1.0)
        nc.vector.memset(s6[:, :, 5:6], 0.0)
        return c6, s6

    def rotate_supertile(s, aM_s, wM_s, RMB_s, c6, s6):
        """Apply y-rotation with cos=c6, sin_th=-s6 (th=-phw); DMA outputs."""
        sl = slice(s * TS, (s + 1) * TS)
        # vectors aM, wM: out0 = c*v0 - s*v2 ; out1 = v1 ; out2 = s*v0 + c*v2
        for src, dst_d, tag in ((aM_s, o_aMc, "a"), (wM_s, o_wMc, "w")):
            out_v = angp.tile([128, TS, 6, 3], FP32, tag=f"rot{tag}")
            t1 = angp.tile([128, TS, 6], FP32, tag="rv1")
            t2 = angp.tile([128, TS, 6], FP32, tag="rv2")
            v0, v1, v2 = src[:, :, :, 0], src[:, :, :, 1], src[:, :, :, 2]
            nc.vector.tensor_tensor(out=t1[:], in0=c6[:], in1=v0, op=A.mult)
            nc.vector.tensor_tensor(out=t2[:], in0=s6[:], in1=v2, op=A.mult)
            nc.vector.tensor_tensor(out=out_v[:, :, :, 0], in0=t1[:], in1=t2[:], op=A.subtract)
            nc.vector.tensor_copy(out=out_v[:, :, :, 1], in_=v1)
            nc.vector.tensor_tensor(out=t1[:], in0=s6[:], in1=v0, op=A.mult)
            nc.vector.tensor_tensor(out=t2[:], in0=c6[:], in1=v2, op=A.mult)
            nc.vector.tensor_tensor(out=out_v[:, :, :, 2], in0=t1[:], in1=t2[:], op=A.add)
            nc.sync.dma_start(out=dst_d.rearrange("(p t) n j -> p t n j", p=128)[:, sl],
                              in_=out_v[:])
        # matrices RMB: row_i transforms, each row is [3] over k
        sh = (128, TS, 6, 3)
        c6b = c6[:].unsqueeze(3).to_broadcast(sh)
        s6b = s6[:].unsqueeze(3).to_broadcast(sh)
        out_m = angp.tile([128, TS, 6, 3, 3], FP32, tag="rotR")
        m1 = angp.tile([128, TS, 6, 3], FP32, tag="rm1")
        m2 = angp.tile([128, TS, 6, 3], FP32, tag="rm2")
        R0, R1, R2 = RMB_s[:, :, :, 0, :], RMB_s[:, :, :, 1, :], RMB_s[:, :, :, 2, :]
        nc.vector.tensor_tensor(out=m1[:], in0=c6b, in1=R0, op=A.mult)
        nc.vector.tensor_tensor(out=m2[:], in0=s6b, in1=R2, op=A.mult)
        nc.vector.tensor_tensor(out=out_m[:, :, :, 0, :], in0=m1[:], in1=m2[:], op=A.subtract)
        nc.vector.tensor_copy(out=out_m[:, :, :, 1, :], in_=R1)
        nc.vector.tensor_tensor(out=m1[:], in0=s6b, in1=R0, op=A.mult)
        nc.vector.tensor_tensor(out=m2[:], in0=c6b, in1=R2, op=A.mult)
        nc.vector.tensor_tensor(out=out_m[:, :, :, 2, :], in0=m1[:], in1=m2[:], op=A.add)
        nc.sync.dma_start(out=o_RMBc.rearrange("(p t) n j k -> p t n j k", p=128)[:, sl],
                          in_=out_m[:])

    # ---------------- main schedule ----------------
    st_tiles = {}
    data_tiles = {}
    for s in range(ST):
        aM_s, wM_s, RMB_s, data_bm = prep_supertile(s)
        st_tiles[s] = (aM_s, wM_s, RMB_s)
        data_tiles[s] = data_bm
        for bt in range(s * (NBT // ST), (s + 1) * (NBT // ST)):
            lstm_batchtile(data_tiles, bt)
        c6, s6 = angle_supertile(s)
        rotate_supertile(s, *st_tiles[s], c6, s6)


def _get_module():
    if 'nc' not in _CACHE:
        _CACHE['nc'] = _build()
    return _CACHE['nc']


def kernel(**inputs):
    inp = {k: np.asarray(v) for k, v in inputs.items()}
    use_flag = int(np.asarray(inp.get('use_flag', 1)))
    delta = np.float32(1.0 / 90.0) if use_flag else np.float32(-1.0 / 90.0)

    nc = _get_module()

    rep = {}
    for name in ('W1', 'b1', 'Wih0', 'bih0', 'bhh0', 'Wih1', 'bih1', 'bhh1', 'W2', 'b2'):
        rep[name] = np.ascontiguousarray(inp[name], dtype=np.float32)
    rep['delta'] = np.array([delta], dtype=np.float32)

    in_maps = []
    for c in range(N_CORES):
        sl = slice(c * BC, (c + 1) * BC)
        m = dict(rep)
        m['aM'] = np.ascontiguousarray(inp['aM'][sl], dtype=np.float32)
        m['wM'] = np.ascontiguousarray(inp['wM'][sl], dtype=np.float32)
        m['RMB'] = np.ascontiguousarray(inp['RMB'][sl], dtype=np.float32)
        m['weight'] = np.ascontiguousarray(inp['weight'][sl], dtype=np.float32)
        in_maps.append(m)

    res = run_bass_kernel_spmd(nc, in_maps, core_ids=list(range(N_CORES)))
    kernel._last_result = res

    aMc = np.concatenate([res.results[c]['aMc'] for c in range(N_CORES)], axis=0)
    wMc = np.concatenate([res.results[c]['wMc'] for c in range(N_CORES)], axis=0)
    RMBc = np.concatenate([res.results[c]['RMBc'] for c in range(N_CORES)], axis=0)
    return aMc, wMc, RMBc
